# revision 34
# baseline (speedup 1.0000x reference)
"""DeepFM (embedding_lookup) Trainium2 kernel — ap_gather edition.

Pure data parallel over B*G rows across 8 NeuronCores. Per core (16K rows):

- Embedding tables live in SBUF, transposed: tabA[16g+j, v] = emb2d[g, v, j]
  (fields 0-7), tabB for fields 8-15. Values are bit-packed u32:
  high16 = bf16(e2[g,v,j]); low16 = bf16(e1d[g,v]) at j==0, bf16(sum_d e2^2)
  at j==1, zero otherwise. One gpsimd ap_gather per (half, 2048-row block)
  yields xT tiles directly (features on partitions, rows on free dim) —
  no DMA descriptors, no PE transposes. Measured: the ap_gather ucode costs
  ~27ns/idx (~32 Q7 cycles; scalar send/clamp/reset_write loops dominate), so
  16 calls x 2048 idxs = ~880us serial on the Pool engine = the critical
  path. Compute (~230us PE) and all DMAs hide under it. Next lever would be
  a faster gather ucode or host-generated DMA descriptors, not scheduling.
- Riders are extracted with one DVE shift-left-16 per tile (partitions with
  zero low16 shift to exactly 0.0), and enter the output matmul via a
  sparse lhsT wE (rows 16g -> wo[0], 16g+1 -> -0.5*wo[1]).
- FM algebra folded into the output matmul as in the dma_gather version:
  s1 via a selection matmul + Square; n@dense_w and s1sq ride one combined
  [29, N] rhs tile; dense_b*wo[0] folds into the output bias.
- DNN runs transposed with f32r matmuls at N=512 (full rate).
"""

import os
import sys

sys.path.insert(0, "/opt/trn_rl_repo")

BISECT = os.environ.get("K_BISECT", "")

from contextlib import ExitStack

import numpy as np

import concourse.tile as tile
from concourse import bacc, mybir
from concourse.bass_utils import run_bass_kernel_spmd
from concourse import library_config

B, G = 128, 1024
F, V, D = 16, 10000, 16
FN = 13
H1, H2, OUT = 256, 128, 128
NCORES = 8
ROWS = B * G
R = ROWS // NCORES  # rows per core

BLK = 2048
NBLK = R // BLK
TN = 512  # rows per compute tile
TPB = BLK // TN

F32 = mybir.dt.float32
F32R = mybir.dt.float32r
I32 = mybir.dt.int32
I16 = mybir.dt.int16
BF16 = mybir.dt.bfloat16
RELU = mybir.ActivationFunctionType.Relu
SQUARE = mybir.ActivationFunctionType.Square
SHL = mybir.AluOpType.logical_shift_left

# Packed weights layout: name -> (col offset, n cols, n valid rows)
WOFFS = {
    "w1A": (0, 256, 128),
    "w1B": (256, 256, 128),
    "w1n": (512, 256, 13),
    "sel": (768, 16, 128),
    "wE": (784, 128, 128),
    "w2c0": (912, 128, 128),
    "w2c1": (1040, 128, 128),
    "wcombo": (1168, 128, 48),
    "wo2": (1296, 128, 128),
}
WCOLS = 1432  # 1424 weight cols + 2 (b1h) + 1 (b2c) + 1 (boc) + 4 align pad


def _pin_swdge_sems_to_queues():
    """Pin each Pool DMA's DMASW sem lane to its queue_num (instead of Tile's
    global round-robin) so multi-queue SWDGE completion sems are consistent."""
    import concourse.tile_sem_assignment as tsa

    if getattr(tsa.TileClockTick, "_qpinned", False):
        return
    orig = tsa.TileClockTick._assign_tick

    def patched(self, inst):
        if isinstance(inst, tsa.DMAInst) and inst.engine == mybir.EngineType.Pool:
            self.next_sw_dma_idx = getattr(inst, "queue_num", 0) or 0
        return orig(self, inst)

    tsa.TileClockTick._assign_tick = patched
    tsa.TileClockTick._qpinned = True


def build_program():
    nc = bacc.Bacc("TRN2", target_bir_lowering=False, debug=False, num_swdge_queues=4)

    tabA_d = nc.dram_tensor("tabA", [128, V], F32, kind="ExternalInput").ap()
    tabB_d = nc.dram_tensor("tabB", [128, V], F32, kind="ExternalInput").ap()
    idxA_d = nc.dram_tensor("idxA", [128, R // 16], I16, kind="ExternalInput").ap()
    idxB_d = nc.dram_tensor("idxB", [128, R // 16], I16, kind="ExternalInput").ap()
    nT_d = nc.dram_tensor("nT", [FN, R], F32R, kind="ExternalInput").ap()
    wpack_d = nc.dram_tensor("wpack", [128, WCOLS], F32R, kind="ExternalInput").ap()
    zer_d = nc.dram_tensor("zer", [19, BLK], F32R, kind="ExternalInput").ap()
    outT_d = nc.dram_tensor("outT", [128, R], F32, kind="ExternalOutput").ap()

    with tile.TileContext(nc) as tc, ExitStack() as ctx:
        singles = ctx.enter_context(tc.tile_pool(name="singles", bufs=1))
        gp = ctx.enter_context(tc.tile_pool(name="gather", bufs=2))
        ntp = ctx.enter_context(tc.tile_pool(name="ntp", bufs=2))
        op = ctx.enter_context(tc.tile_pool(name="outbuf", bufs=2))
        rp = ctx.enter_context(tc.tile_pool(name="riders", bufs=2))
        xp = ctx.enter_context(tc.tile_pool(name="xt", bufs=2))
        h1p = ctx.enter_context(tc.tile_pool(name="h1p", bufs=2))
        h2p = ctx.enter_context(tc.tile_pool(name="h2p", bufs=2))
        ps_h1 = ctx.enter_context(tc.tile_pool(name="ps_h1", bufs=2, space="PSUM"))
        ps_h2 = ctx.enter_context(tc.tile_pool(name="ps_h2", bufs=1, space="PSUM"))
        ps_wo = ctx.enter_context(tc.tile_pool(name="ps_wo", bufs=1, space="PSUM"))
        ps_s1 = ctx.enter_context(tc.tile_pool(name="ps_s1", bufs=1, space="PSUM"))

        nc.gpsimd.load_library(library_config.ap_gather)

        idxA = singles.tile([128, R // 16], I16)
        nc.sync.dma_start(out=idxA[:], in_=idxA_d[:])
        tabA = singles.tile([128, V], F32)
        nc.gpsimd.dma_start(out=tabA[:], in_=tabA_d[:])
        idxB = singles.tile([128, R // 16], I16)
        nc.sync.dma_start(out=idxB[:], in_=idxB_d[:])
        tabB = singles.tile([128, V], F32)
        nc.gpsimd.dma_start(out=tabB[:], in_=tabB_d[:])
        wpk = singles.tile([128, WCOLS], F32R)
        nc.sync.dma_start(out=wpk[:], in_=wpack_d[:])

        wt = {
            name: wpk[0:nrows, off : off + ncols]
            for name, (off, ncols, nrows) in WOFFS.items()
        }
        b1h = wpk[:, 1424:1426].bitcast(F32)
        b2c = wpk[:, 1426:1427].bitcast(F32)
        boc = wpk[:, 1427:1428].bitcast(F32)

        # Priming ops: absorb input-DMA semaphores into each engine's program
        # order so steady-state instructions carry few sync waits.
        warm = ps_wo.tile([16, 16], F32, space="PSUM", tag="warm")
        nc.tensor.matmul(warm[:], wt["sel"][:], wpk[:, 0:16].bitcast(F32R), start=True, stop=True)
        scr = singles.tile([128, 1], F32, tag="scr")
        nc.scalar.activation(out=scr[:], in_=b1h[:, 0:1], func=mybir.ActivationFunctionType.Copy)
        scr2 = singles.tile([128, 1], F32, tag="scr2")
        nc.vector.tensor_copy(out=scr2[:], in_=b2c[:, 0:1])
        # Zero rows 13:32 of both ntb buffers once: steady state only writes
        # rows 0:13 (nT DMA) and 32:48 (s1sq), so the wcombo matmul must see
        # 0.0 in between, never garbage.
        for _ in range(2):
            cz = ntp.tile([48, BLK], F32R, tag="ntb")
            nc.sync.dma_start(out=cz[13:32, :], in_=zer_d[:])

        for b in range(NBLK):
            islc = slice(b * (BLK // 16), (b + 1) * (BLK // 16))
            gA = gp.tile([128, BLK], F32, tag="gA")
            nc.gpsimd.ap_gather(
                out_ap=gA[:], in_ap=tabA[:], idxs_ap=idxA[:, islc],
                channels=128, num_elems=V, d=1, num_idxs=BLK,
            )
            gB = gp.tile([128, BLK], F32, tag="gB")
            nc.gpsimd.ap_gather(
                out_ap=gB[:], in_ap=tabB[:], idxs_ap=idxB[:, islc],
                channels=128, num_elems=V, d=1, num_idxs=BLK,
            )
            if BISECT == "gather_only":
                nc.gpsimd.dma_start(out=outT_d[:, b * BLK : (b + 1) * BLK], in_=gA[:])
                continue
            # riders: low bf16 half of each u32 word (element 0 of each pair)
            xtAf = xp.tile([128, BLK], F32R, tag="xA")
            nc.vector.tensor_copy(out=xtAf[:], in_=gA[:].bitcast(F32R))
            xtBf = xp.tile([128, BLK], F32R, tag="xB")
            nc.vector.tensor_copy(out=xtBf[:], in_=gB[:].bitcast(F32R))
            outb = op.tile([128, BLK], F32)
            nTb = ntp.tile([48, BLK], F32R, tag="ntb")
            nc.sync.dma_start(out=nTb[0:FN, :], in_=nT_d[:, b * BLK : (b + 1) * BLK])

            for t in range(TPB):
                sl = slice(t * TN, (t + 1) * TN)
                rlo = b * BLK + t * TN
                xtA = xtAf[:, sl]
                xtB = xtBf[:, sl]

                if BISECT != "noriders":
                    rAb = rp.tile([128, TN], F32R, tag="rA")
                    nc.vector.tensor_copy(
                        out=rAb[:],
                        in_=gA[:, sl].bitcast(BF16).rearrange("p (n two) -> p n two", two=2)[:, :, 0:1].rearrange("p n one -> p (n one)"),
                    )
                    rBb = rp.tile([128, TN], F32R, tag="rB")
                    nc.vector.tensor_copy(
                        out=rBb[:],
                        in_=gB[:, sl].bitcast(BF16).rearrange("p (n two) -> p n two", two=2)[:, :, 0:1].rearrange("p n one -> p (n one)"),
                    )

                # s1 = sum_f e_f (transposed [16, TN]) -> squared
                ps1 = ps_s1.tile([16, TN], F32, space="PSUM")
                nc.tensor.matmul(ps1[:], wt["sel"][:], xtA, start=True, stop=False)
                nc.tensor.matmul(ps1[:], wt["sel"][:], xtB, start=False, stop=True)
                nc.scalar.activation(out=nTb[32:48, t * TN : (t + 1) * TN], in_=ps1[:], func=SQUARE)

                # h1 = relu(x @ w1 + b1) -> h1T [256, TN] in 2 halves
                ph1 = ps_h1.tile([128, 2 * TN], F32, space="PSUM")
                for mh in range(2):
                    dst = ph1[:, mh * TN : (mh + 1) * TN]
                    ms = slice(mh * 128, (mh + 1) * 128)
                    nc.tensor.matmul(dst, wt["w1A"][:, ms], xtA, start=True, stop=False)
                    nc.tensor.matmul(dst, wt["w1B"][:, ms], xtB, start=False, stop=False)
                    nc.tensor.matmul(dst, wt["w1n"][:, ms], nTb[0:FN, t * TN : (t + 1) * TN], start=False, stop=True)
                h1t = h1p.tile([128, 2 * TN], F32R)
                nc.scalar.activation(out=h1t[:, 0:TN], in_=ph1[:, 0:TN], func=RELU, bias=b1h[:, 0:1])
                nc.scalar.activation(out=h1t[:, TN : 2 * TN], in_=ph1[:, TN : 2 * TN], func=RELU, bias=b1h[:, 1:2])

                # h2 = relu(h1 @ w2 + b2) -> h2T [128, TN]
                ph2 = ps_h2.tile([128, TN], F32, space="PSUM")
                nc.tensor.matmul(ph2[:], wt["w2c0"][:], h1t[:, 0:TN], start=True, stop=False)
                nc.tensor.matmul(ph2[:], wt["w2c1"][:], h1t[:, TN : 2 * TN], start=False, stop=True)
                h2t = h2p.tile([128, TN], F32R)
                nc.vector.tensor_scalar(
                    out=h2t[:], in0=ph2[:], scalar1=b2c[:, 0:1],
                    scalar2=0.0, op0=mybir.AluOpType.add, op1=mybir.AluOpType.max,
                )

                # out = relu(pe @ wo + bo') -> outT [128, TN]
                pwo = ps_wo.tile([128, TN], F32, space="PSUM")
                if BISECT != "noriders":
                    nc.tensor.matmul(pwo[:], wt["wE"][:], rAb[:], start=True, stop=False)
                    nc.tensor.matmul(pwo[:], wt["wE"][:], rBb[:], start=False, stop=False)
                    nc.tensor.matmul(pwo[:], wt["wcombo"][:], nTb[:, t * TN : (t + 1) * TN], start=False, stop=False)
                else:
                    nc.tensor.matmul(pwo[:], wt["wcombo"][:], nTb[:, t * TN : (t + 1) * TN], start=True, stop=False)
                nc.tensor.matmul(pwo[:], wt["wo2"][:], h2t[:], start=False, stop=True)
                nc.vector.tensor_scalar(
                    out=outb[:, sl], in0=pwo[:], scalar1=boc[:, 0:1],
                    scalar2=0.0, op0=mybir.AluOpType.add, op1=mybir.AluOpType.max,
                )

            nc.scalar.dma_start(out=outT_d[:, b * BLK : (b + 1) * BLK], in_=outb[:])

    nc.compile()
    return nc


def _bf16_bits(x):
    """f32 ndarray -> round-to-nearest-even bf16 bit pattern as uint32."""
    u = np.ascontiguousarray(x, np.float32).view(np.uint32)
    return ((u.astype(np.uint64) + 0x7FFF + ((u >> 16) & 1)) >> 16).astype(np.uint32)


def prep_shared(inputs):
    emb1d = np.asarray(inputs["emb1d"], np.float32)
    emb2d = np.asarray(inputs["emb2d"], np.float32)
    dense_w = np.asarray(inputs["dense_w"], np.float32)
    dense_b = np.asarray(inputs["dense_b"], np.float32)
    w1 = np.asarray(inputs["w1"], np.float32)
    b1 = np.asarray(inputs["b1"], np.float32)
    w2 = np.asarray(inputs["w2"], np.float32)
    b2 = np.asarray(inputs["b2"], np.float32)
    wo = np.asarray(inputs["wo"], np.float32)
    bo = np.asarray(inputs["bo"], np.float32)

    sumsq = (emb2d.astype(np.float64) ** 2).sum(-1).astype(np.float32)  # [F, V]
    tabs = np.zeros((2, 128, V), np.uint32)
    for h in range(2):
        for g in range(8):
            f = h * 8 + g
            hi = _bf16_bits(emb2d[f].T) << 16  # [16, V]
            hi[0] |= _bf16_bits(emb1d[f, :, 0])
            hi[1] |= _bf16_bits(sumsq[f])
            tabs[h, g * 16 : g * 16 + 16] = hi

    blocks = dict(
        w1A=w1[0:128],
        w1B=w1[128:256],
        w1n=w1[256:269],
        sel=np.tile(np.eye(16, dtype=np.float32), (8, 1)),
        wE=np.zeros((128, 128), np.float32),
        w2c0=w2[0:128],
        w2c1=w2[128:256],
        wcombo=np.concatenate(
            [dense_w @ wo[0:1], np.zeros((19, OUT), np.float32), np.tile(0.5 * wo[1:2], (D, 1))], 0
        ),
        wo2=wo[2:130],
    )
    idx16 = np.arange(8) * 16
    blocks["wE"][idx16] = wo[0]
    blocks["wE"][idx16 + 1] = -0.5 * wo[1]

    wpack = np.zeros((128, WCOLS), np.float32)
    for name, (off, ncols, nrows) in WOFFS.items():
        arr = blocks[name]
        assert arr.shape == (nrows, ncols), (name, arr.shape, (nrows, ncols))
        wpack[0:nrows, off : off + ncols] = arr
    wpack[:, 1424] = b1[0:128]
    wpack[:, 1425] = b1[128:256]
    wpack[:, 1426] = b2
    wpack[:, 1427] = bo + dense_b[0] * wo[0]
    return dict(tabA=tabs[0].view(np.float32), tabB=tabs[1].view(np.float32), wpack=wpack,
                zer=np.zeros((19, BLK), np.float32))


def make_core_inputs(shared, cf, nflat, rlo):
    cb = cf[rlo : rlo + R]  # [R, F]
    idx = np.empty((2, 128, R // 16), np.int16)
    for h in range(2):
        for g in range(8):
            idx[h, 16 * g : 16 * g + 16] = cb[:, h * 8 + g].reshape(R // 16, 16).T
    m = dict(shared)
    m["idxA"] = np.ascontiguousarray(idx[0])
    m["idxB"] = np.ascontiguousarray(idx[1])
    m["nT"] = np.ascontiguousarray(nflat[rlo : rlo + R].T)
    return m


_NC_CACHE = {}
_RUN_KWARGS = {}


def kernel(**inputs):
    n_features = np.asarray(inputs["n_features"], np.float32)
    c_features = np.asarray(inputs["c_features"])

    shared = prep_shared(inputs)
    cf = c_features.reshape(ROWS, F).astype(np.int32)
    nflat = n_features.reshape(ROWS, FN)

    if "nc" not in _NC_CACHE:
        _NC_CACHE["nc"] = build_program()
    nc = _NC_CACHE["nc"]

    in_maps = [make_core_inputs(shared, cf, nflat, k * R) for k in range(NCORES)]
    kres = run_bass_kernel_spmd(nc, in_maps, list(range(NCORES)), **_RUN_KWARGS)
    kernel.last_results = kres
    res = kres.results

    out = np.empty((ROWS, OUT), np.float32)
    for k in range(NCORES):
        out[k * R : (k + 1) * R] = res[k]["outT"].T
    return out.reshape(B, G, OUT)


if __name__ == "__main__":
    print("building program...")
    nc = build_program()
    print("ok")
